# revision 20
# baseline (speedup 1.0000x reference)
"""DGM cell stack (3 layers) on 8 NeuronCores, data-parallel over batch.

Layout: activations transposed to [H=128 partitions, batch free dim].
All biases are folded into the matmuls via an augmented ones-row on x.
The G gate's weights are pre-negated on the host so sigmoid gives (1-G)
directly, letting Z/(1-G)/R share one merged sigmoid over 3 PSUM banks.
Matmul operands are bf16 (fast weight load path); accumulation is fp32
and the recurrent state S stays fp32 so rounding does not compound.
Emission is software-pipelined across batch tiles so all engines overlap.
"""

import os

import numpy as np
import ml_dtypes

import concourse.bacc as bacc
import concourse.tile as tile
from concourse import mybir
from concourse.bass_utils import run_bass_kernel_spmd

B, D_IN, H, N_LAYERS = 262144, 8, 128, 3
N_CORES = 8
B_CORE = B // N_CORES
N = 512                     # batch columns per tile (one PSUM bank of fp32)
N_TILES = B_CORE // N
KA = D_IN + 1               # augmented contraction dim (ones row + x rows)

f32 = mybir.dt.float32
bf16 = mybir.dt.bfloat16
SIG = mybir.ActivationFunctionType.Sigmoid
TANH = mybir.ActivationFunctionType.Tanh

_CACHE = {}


def _build(reps=1):
    nc = bacc.Bacc("TRN2", target_bir_lowering=False)

    xa_d = nc.dram_tensor("xa", [KA, B_CORE], bf16, kind="ExternalInput")
    sw_d = nc.dram_tensor("sw", [KA, H], bf16, kind="ExternalInput")
    # u: [layer, gate(z,g,r,h), KA, H] augmented input-side weights (bias row 0)
    u_d = nc.dram_tensor("u", [N_LAYERS, 4, KA, H], bf16, kind="ExternalInput")
    # w: [layer, gate(z,g,r,h), H, H] state-side weights
    w_d = nc.dram_tensor("w", [N_LAYERS, 4, H, H], bf16, kind="ExternalInput")
    wf_d = nc.dram_tensor("wf", [H, 1], bf16, kind="ExternalInput")
    wfb_d = nc.dram_tensor("wfb", [1, 1], f32, kind="ExternalInput")
    out_d = nc.dram_tensor("out", [1, B_CORE], f32, kind="ExternalOutput")

    bufs_small = int(os.environ.get("K_BUFS_SMALL", "8"))
    bufs_zgr_sb = int(os.environ.get("K_BUFS_ZGRSB", "3"))
    bufs_xa = int(os.environ.get("K_BUFS_XA", "6"))
    bufs_ps_zgr = int(os.environ.get("K_BUFS_PSZGR", "2"))
    bufs_ps_h = int(os.environ.get("K_BUFS_PSH", "2"))

    with tile.TileContext(nc) as tc:
        with (
            tc.tile_pool(name="consts", bufs=1) as cp,
            tc.tile_pool(name="small", bufs=bufs_small) as sp,
            tc.tile_pool(name="zgrsb", bufs=bufs_zgr_sb) as zp,
            tc.tile_pool(name="xap", bufs=bufs_xa) as xp,
            tc.tile_pool(name="ps_zgr", bufs=bufs_ps_zgr, space="PSUM") as pz,
            tc.tile_pool(name="ps_small", bufs=bufs_ps_h, space="PSUM") as pm,
        ):
            t_sw = cp.tile([KA, H], bf16)
            nc.sync.dma_start(out=t_sw[:], in_=sw_d[:])
            t_u = cp.tile([KA, N_LAYERS * 4 * H], bf16)
            t_w = cp.tile([H, N_LAYERS * 4 * H], bf16)
            for l in range(N_LAYERS):
                for g in range(4):
                    off = (l * 4 + g) * H
                    nc.sync.dma_start(out=t_u[:, off:off + H], in_=u_d[l, g])
                    nc.sync.dma_start(out=t_w[:, off:off + H], in_=w_d[l, g])
            t_wf = cp.tile([H, 1], bf16)
            nc.sync.dma_start(out=t_wf[:], in_=wf_d[:])
            t_wfb = cp.tile([1, 1], f32)
            nc.sync.dma_start(out=t_wfb[:], in_=wfb_d[:])

            def u_ap(layer, gate):
                off = (layer * 4 + gate) * H
                return t_u[:, off:off + H]

            def w_ap(layer, gate):
                off = (layer * 4 + gate) * H
                return t_w[:, off:off + H]

            state = {}

            def stage_load(t):
                xa = xp.tile([KA, N], bf16, tag="xa")
                nc.sync.dma_start(out=xa[:], in_=xa_d[:, t * N:(t + 1) * N])
                state[t] = {"xa": xa, "s": None, "ps_h": None}

            def stage_layer(t, l):
                st = state[t]
                xa = st["xa"]
                if l == 0:
                    ps_s0 = pm.tile([H, N], f32, tag="h")
                    nc.tensor.matmul(ps_s0[:], t_sw[:], xa[:],
                                     start=True, stop=True)
                s_prev = st["s"]

                a = sp.tile([H, N], bf16, tag="a")
                nc.scalar.activation(a[:], ps_s0[:] if l == 0 else s_prev[:], SIG)

                # gate order in psum slots: r, z, g (r first: its sigmoid can
                # split out early to start the H chain)
                ps_zgr = pz.tile([H, 3 * N], f32, tag="zgr")
                ps_h = pm.tile([H, N], f32, tag="h")
                order = [2, 0, 1]
                for slot, g in enumerate(order):
                    nc.tensor.matmul(ps_zgr[:, slot * N:(slot + 1) * N],
                                     u_ap(l, g), xa[:], start=True, stop=False)
                nc.tensor.matmul(ps_h[:], u_ap(l, 3), xa[:],
                                 start=True, stop=False)
                for slot, g in enumerate(order):
                    nc.tensor.matmul(ps_zgr[:, slot * N:(slot + 1) * N],
                                     w_ap(l, g), a[:], start=False, stop=True)

                zgr = zp.tile([H, 3 * N], f32, tag="zgr_sb")
                if os.environ.get("K_SPLIT_R", "0") == "1":
                    nc.scalar.activation(zgr[:, 0:N], ps_zgr[:, 0:N], SIG)
                    nc.scalar.activation(zgr[:, N:3 * N], ps_zgr[:, N:3 * N], SIG)
                else:
                    nc.scalar.activation(zgr[:], ps_zgr[:], SIG)
                r_sl = zgr[:, 0:N]
                z_sl = zgr[:, N:2 * N]
                g_sl = zgr[:, 2 * N:3 * N]

                sr = sp.tile([H, N], bf16, tag="sr")
                nc.vector.tensor_mul(sr[:], a[:], r_sl)
                nc.tensor.matmul(ps_h[:], w_ap(l, 3), sr[:],
                                 start=False, stop=True)

                hh = sp.tile([H, N], f32, tag="hh")
                nc.scalar.activation(hh[:], ps_h[:], TANH)

                t1 = sp.tile([H, N], f32, tag="t1")
                nc.vector.tensor_mul(t1[:], g_sl, hh[:])        # (1-G)*H
                t2 = sp.tile([H, N], f32, tag="t2")
                nc.gpsimd.tensor_mul(t2[:], z_sl, a[:])         # Z*A
                if l < N_LAYERS - 1:
                    s_new = sp.tile([H, N], f32, tag="s")
                else:
                    # last layer: S feeds only the final matmul -> bf16
                    s_new = sp.tile([H, N], bf16, tag="s3")
                nc.vector.tensor_add(s_new[:], t1[:], t2[:])
                st["s"] = s_new
                st["ps_h"] = ps_h

            def stage_out(t):
                st = state.pop(t)
                s3, ps_h = st["s"], st["ps_h"]
                # final projection rides in rows [0:1] of the (consumed) ps_h
                ps_out = ps_h[0:1, :]
                nc.tensor.matmul(ps_out, t_wf[:], s3[:], start=True, stop=True)
                o_sb = sp.tile([1, N], f32, tag="o_sb")
                nc.vector.tensor_scalar_add(o_sb[:], ps_out, t_wfb[:])
                nc.sync.dma_start(out=out_d[0:1, t * N:(t + 1) * N], in_=o_sb[:])

            # software-pipelined emission, deepest stage first
            SPC = int(os.environ.get("K_SPACING", "2"))
            DEPTH = 1 + N_LAYERS * SPC + 1

            def body(_=None):
                for k in range(N_TILES + DEPTH):
                    if 0 <= k - DEPTH:
                        stage_out(k - DEPTH)
                    for l in reversed(range(N_LAYERS)):
                        ti = k - 1 - l * SPC
                        if 0 <= ti < N_TILES:
                            stage_layer(ti, l)
                    if k < N_TILES:
                        stage_load(k)

            if reps == 1:
                body()
            else:
                with tc.For_i(0, reps, 1) as _i:
                    body(_i)

    nc.compile()
    return nc


def _prep(inputs):
    x = np.asarray(inputs["x"], np.float32)
    xaug = np.empty((KA, B), np.float32)
    xaug[0] = 1.0
    xaug[1:] = x.T

    sw = np.concatenate(
        [np.asarray(inputs["Sw_b"], np.float32)[None, :],
         np.asarray(inputs["Sw_w"], np.float32)], axis=0)

    u = np.empty((N_LAYERS, 4, KA, H), np.float32)
    w = np.empty((N_LAYERS, 4, H, H), np.float32)
    gates_u = ["Uz", "Ug", "Ur", "Uh"]
    gates_wb = [("Wsz_w", "Wsz_b"), ("Wsg_w", "Wsg_b"),
                ("Wsr_w", "Wsr_b"), ("Wsh_w", "Wsh_b")]
    for g in range(4):
        u[:, g, 0, :] = np.asarray(inputs[gates_wb[g][1]], np.float32)
        u[:, g, 1:, :] = np.asarray(inputs[gates_u[g]], np.float32)
        w[:, g] = np.asarray(inputs[gates_wb[g][0]], np.float32)
    # negate the G gate so sigmoid(pre) = 1 - G
    u[:, 1] *= -1.0
    w[:, 1] *= -1.0

    wf = np.asarray(inputs["Wf_w"], np.float32).reshape(H, 1)
    wfb = np.asarray(inputs["Wf_b"], np.float32).reshape(1, 1)

    b = ml_dtypes.bfloat16
    return (xaug.astype(b), sw.astype(b), u.astype(b), w.astype(b),
            wf.astype(b), wfb)


def kernel(**inputs):
    if "nc" not in _CACHE:
        _CACHE["nc"] = _build()
    nc = _CACHE["nc"]

    xaug, sw, u, w, wf, wfb = _prep(inputs)
    in_maps = []
    for c in range(N_CORES):
        in_maps.append({
            "xa": np.ascontiguousarray(xaug[:, c * B_CORE:(c + 1) * B_CORE]),
            "sw": sw, "u": u, "w": w, "wf": wf, "wfb": wfb,
        })
    res = run_bass_kernel_spmd(nc, in_maps, core_ids=list(range(N_CORES)))
    out = np.concatenate([res.results[c]["out"][0] for c in range(N_CORES)])
    return out.reshape(B, 1).astype(np.float32)


# revision 21
# speedup vs baseline: 1.0308x; 1.0308x over previous
"""DGM cell stack (3 layers) on 8 NeuronCores, data-parallel over batch.

Layout: activations transposed to [H=128 partitions, batch free dim].
All biases are folded into the matmuls via an augmented ones-row on x.
The G gate's weights are pre-negated on the host so sigmoid gives (1-G)
directly, letting Z/(1-G)/R share one merged sigmoid over 3 PSUM banks.
Matmul operands are bf16 (fast weight load path); accumulation is fp32
and the recurrent state S stays fp32 so rounding does not compound.
Emission is software-pipelined across batch tiles so all engines overlap.
"""

import os

import numpy as np
import ml_dtypes

import concourse.bacc as bacc
import concourse.tile as tile
from concourse import mybir
from concourse.bass_utils import run_bass_kernel_spmd

B, D_IN, H, N_LAYERS = 262144, 8, 128, 3
N_CORES = 8
B_CORE = B // N_CORES
N = 512                     # batch columns per tile (one PSUM bank of fp32)
N_TILES = B_CORE // N
KA = D_IN + 1               # augmented contraction dim (ones row + x rows)

f32 = mybir.dt.float32
bf16 = mybir.dt.bfloat16
SIG = mybir.ActivationFunctionType.Sigmoid
TANH = mybir.ActivationFunctionType.Tanh

_CACHE = {}


def _build(reps=1):
    nc = bacc.Bacc("TRN2", target_bir_lowering=False)

    xa_d = nc.dram_tensor("xa", [KA, B_CORE], bf16, kind="ExternalInput")
    sw_d = nc.dram_tensor("sw", [KA, H], bf16, kind="ExternalInput")
    # u: [layer, gate(z,g,r,h), KA, H] augmented input-side weights (bias row 0)
    u_d = nc.dram_tensor("u", [N_LAYERS, 4, KA, H], bf16, kind="ExternalInput")
    # w: [layer, gate(z,g,r,h), H, H] state-side weights
    w_d = nc.dram_tensor("w", [N_LAYERS, 4, H, H], bf16, kind="ExternalInput")
    wf_d = nc.dram_tensor("wf", [H, 1], bf16, kind="ExternalInput")
    wfb_d = nc.dram_tensor("wfb", [1, 1], f32, kind="ExternalInput")
    out_d = nc.dram_tensor("out", [1, B_CORE], f32, kind="ExternalOutput")

    bufs_small = int(os.environ.get("K_BUFS_SMALL", "8"))
    bufs_zgr_sb = int(os.environ.get("K_BUFS_ZGRSB", "3"))
    bufs_xa = int(os.environ.get("K_BUFS_XA", "6"))
    bufs_ps_zgr = int(os.environ.get("K_BUFS_PSZGR", "2"))
    bufs_ps_h = int(os.environ.get("K_BUFS_PSH", "2"))

    with tile.TileContext(nc) as tc:
        with (
            tc.tile_pool(name="consts", bufs=1) as cp,
            tc.tile_pool(name="small", bufs=bufs_small) as sp,
            tc.tile_pool(name="zgrsb", bufs=bufs_zgr_sb) as zp,
            tc.tile_pool(name="xap", bufs=bufs_xa) as xp,
            tc.tile_pool(name="ps_zgr", bufs=bufs_ps_zgr, space="PSUM") as pz,
            tc.tile_pool(name="ps_small", bufs=bufs_ps_h, space="PSUM") as pm,
        ):
            t_sw = cp.tile([KA, H], bf16)
            nc.sync.dma_start(out=t_sw[:], in_=sw_d[:])
            t_u = cp.tile([KA, N_LAYERS * 4 * H], bf16)
            t_w = cp.tile([H, N_LAYERS * 4 * H], bf16)
            for l in range(N_LAYERS):
                for g in range(4):
                    off = (l * 4 + g) * H
                    nc.sync.dma_start(out=t_u[:, off:off + H], in_=u_d[l, g])
                    nc.sync.dma_start(out=t_w[:, off:off + H], in_=w_d[l, g])
            t_wf = cp.tile([H, 1], bf16)
            nc.sync.dma_start(out=t_wf[:], in_=wf_d[:])
            t_wfb = cp.tile([1, 1], f32)
            nc.sync.dma_start(out=t_wfb[:], in_=wfb_d[:])

            def u_ap(layer, gate):
                off = (layer * 4 + gate) * H
                return t_u[:, off:off + H]

            def w_ap(layer, gate):
                off = (layer * 4 + gate) * H
                return t_w[:, off:off + H]

            ABL = os.environ.get("K_ABL", "")
            if ABL == "mm_only":
                acf = cp.tile([H, N], f32)
                nc.vector.memset(acf[:], 0.5)
                a_const = cp.tile([H, N], bf16)
                nc.vector.tensor_copy(a_const[:], acf[:])
            state = {}

            def stage_load(t):
                xa = xp.tile([KA, N], bf16, tag="xa")
                nc.sync.dma_start(out=xa[:], in_=xa_d[:, t * N:(t + 1) * N])
                state[t] = {"xa": xa, "s": None, "ps_h": None}

            def stage_layer(t, l):
                st = state[t]
                xa = st["xa"]
                if l == 0:
                    ps_s0 = pm.tile([H, N], f32, tag="h")
                    nc.tensor.matmul(ps_s0[:], t_sw[:], xa[:],
                                     start=True, stop=True)
                s_prev = st["s"]

                if ABL == "mm_only":
                    a = a_const
                else:
                    a = sp.tile([H, N], bf16, tag="a")
                    nc.scalar.activation(a[:], ps_s0[:] if l == 0 else s_prev[:], SIG)

                # gate order in psum slots: r, z, g (r first: its sigmoid can
                # split out early to start the H chain)
                ps_zgr = pz.tile([H, 3 * N], f32, tag="zgr")
                ps_h = pm.tile([H, N], f32, tag="h")
                order = [2, 0, 1]
                for slot, g in enumerate(order):
                    nc.tensor.matmul(ps_zgr[:, slot * N:(slot + 1) * N],
                                     u_ap(l, g), xa[:], start=True, stop=False)
                nc.tensor.matmul(ps_h[:], u_ap(l, 3), xa[:],
                                 start=True, stop=False)
                for slot, g in enumerate(order):
                    nc.tensor.matmul(ps_zgr[:, slot * N:(slot + 1) * N],
                                     w_ap(l, g), a[:], start=False, stop=True)

                zgr = zp.tile([H, 3 * N], f32, tag="zgr_sb")
                if ABL == "mm_only":
                    pass
                elif os.environ.get("K_SPLIT_R", "0") == "1":
                    nc.scalar.activation(zgr[:, 0:N], ps_zgr[:, 0:N], SIG)
                    nc.scalar.activation(zgr[:, N:3 * N], ps_zgr[:, N:3 * N], SIG)
                else:
                    nc.scalar.activation(zgr[:], ps_zgr[:], SIG)
                r_sl = zgr[:, 0:N]
                z_sl = zgr[:, N:2 * N]
                g_sl = zgr[:, 2 * N:3 * N]

                if ABL == "mm_only":
                    sr = a_const
                else:
                    sr = sp.tile([H, N], bf16, tag="sr")
                    nc.vector.tensor_mul(sr[:], a[:], r_sl)
                nc.tensor.matmul(ps_h[:], w_ap(l, 3), sr[:],
                                 start=False, stop=True)

                if ABL == "mm_only":
                    st["s"] = a_const
                    st["ps_h"] = ps_h
                    return
                hh = sp.tile([H, N], f32, tag="hh")
                nc.scalar.activation(hh[:], ps_h[:], TANH)

                t1 = sp.tile([H, N], f32, tag="t1")
                nc.vector.tensor_mul(t1[:], g_sl, hh[:])        # (1-G)*H
                t2 = sp.tile([H, N], f32, tag="t2")
                nc.gpsimd.tensor_mul(t2[:], z_sl, a[:])         # Z*A
                if l < N_LAYERS - 1:
                    s_new = sp.tile([H, N], f32, tag="s")
                else:
                    # last layer: S feeds only the final matmul -> bf16
                    s_new = sp.tile([H, N], bf16, tag="s3")
                nc.vector.tensor_add(s_new[:], t1[:], t2[:])
                st["s"] = s_new
                st["ps_h"] = ps_h

            def stage_out(t):
                st = state.pop(t)
                s3, ps_h = st["s"], st["ps_h"]
                # final projection rides in rows [0:1] of the (consumed) ps_h
                ps_out = ps_h[0:1, :]
                nc.tensor.matmul(ps_out, t_wf[:], s3[:], start=True, stop=True)
                o_sb = sp.tile([1, N], f32, tag="o_sb")
                nc.vector.tensor_scalar_add(o_sb[:], ps_out, t_wfb[:])
                nc.sync.dma_start(out=out_d[0:1, t * N:(t + 1) * N], in_=o_sb[:])

            # software-pipelined emission, deepest stage first
            SPC = int(os.environ.get("K_SPACING", "2"))
            DEPTH = 1 + N_LAYERS * SPC + 1

            def body(_=None):
                for k in range(N_TILES + DEPTH):
                    if 0 <= k - DEPTH:
                        stage_out(k - DEPTH)
                    for l in reversed(range(N_LAYERS)):
                        ti = k - 1 - l * SPC
                        if 0 <= ti < N_TILES:
                            stage_layer(ti, l)
                    if k < N_TILES:
                        stage_load(k)

            if reps == 1:
                body()
            else:
                with tc.For_i(0, reps, 1) as _i:
                    body(_i)

    nc.compile()
    return nc


def _prep(inputs):
    x = np.asarray(inputs["x"], np.float32)
    xaug = np.empty((KA, B), np.float32)
    xaug[0] = 1.0
    xaug[1:] = x.T

    sw = np.concatenate(
        [np.asarray(inputs["Sw_b"], np.float32)[None, :],
         np.asarray(inputs["Sw_w"], np.float32)], axis=0)

    u = np.empty((N_LAYERS, 4, KA, H), np.float32)
    w = np.empty((N_LAYERS, 4, H, H), np.float32)
    gates_u = ["Uz", "Ug", "Ur", "Uh"]
    gates_wb = [("Wsz_w", "Wsz_b"), ("Wsg_w", "Wsg_b"),
                ("Wsr_w", "Wsr_b"), ("Wsh_w", "Wsh_b")]
    for g in range(4):
        u[:, g, 0, :] = np.asarray(inputs[gates_wb[g][1]], np.float32)
        u[:, g, 1:, :] = np.asarray(inputs[gates_u[g]], np.float32)
        w[:, g] = np.asarray(inputs[gates_wb[g][0]], np.float32)
    # negate the G gate so sigmoid(pre) = 1 - G
    u[:, 1] *= -1.0
    w[:, 1] *= -1.0

    wf = np.asarray(inputs["Wf_w"], np.float32).reshape(H, 1)
    wfb = np.asarray(inputs["Wf_b"], np.float32).reshape(1, 1)

    b = ml_dtypes.bfloat16
    return (xaug.astype(b), sw.astype(b), u.astype(b), w.astype(b),
            wf.astype(b), wfb)


def kernel(**inputs):
    if "nc" not in _CACHE:
        _CACHE["nc"] = _build()
    nc = _CACHE["nc"]

    xaug, sw, u, w, wf, wfb = _prep(inputs)
    in_maps = []
    for c in range(N_CORES):
        in_maps.append({
            "xa": np.ascontiguousarray(xaug[:, c * B_CORE:(c + 1) * B_CORE]),
            "sw": sw, "u": u, "w": w, "wf": wf, "wfb": wfb,
        })
    res = run_bass_kernel_spmd(nc, in_maps, core_ids=list(range(N_CORES)))
    out = np.concatenate([res.results[c]["out"][0] for c in range(N_CORES)])
    return out.reshape(B, 1).astype(np.float32)


# revision 22
# speedup vs baseline: 1.5865x; 1.5391x over previous
"""DGM cell stack (3 layers) on 8 NeuronCores, data-parallel over batch.

Layout: activations transposed to [H=128 partitions, batch free dim].
All biases are folded into the matmuls via an augmented ones-row on x.
The G gate's weights are pre-negated on the host so sigmoid gives (1-G)
directly, letting Z/(1-G)/R share one merged sigmoid over 3 PSUM banks.
Matmul operands are bf16 (fast weight load path); accumulation is fp32
and the recurrent state S stays fp32 so rounding does not compound.
Emission is software-pipelined across batch tiles so all engines overlap.
"""

import os

import numpy as np
import ml_dtypes

import concourse.bacc as bacc
import concourse.tile as tile
from concourse import mybir
from concourse.bass_utils import run_bass_kernel_spmd

B, D_IN, H, N_LAYERS = 262144, 8, 128, 3
N_CORES = 8
B_CORE = B // N_CORES
N = 512                     # batch columns per tile (one PSUM bank of fp32)
N_TILES = B_CORE // N
KA = D_IN + 1               # augmented contraction dim (ones row + x rows)

f32 = mybir.dt.float32
bf16 = mybir.dt.bfloat16
SIG = mybir.ActivationFunctionType.Sigmoid
TANH = mybir.ActivationFunctionType.Tanh

_CACHE = {}


def _build(reps=1):
    nc = bacc.Bacc("TRN2", target_bir_lowering=False)

    xa_d = nc.dram_tensor("xa", [KA, B_CORE], bf16, kind="ExternalInput")
    sw_d = nc.dram_tensor("sw", [KA, H], bf16, kind="ExternalInput")
    # u: [layer, gate(z,g,r,h), KA, H] augmented input-side weights (bias row 0)
    u_d = nc.dram_tensor("u", [N_LAYERS, 4, KA, H], bf16, kind="ExternalInput")
    # w: [layer, gate(z,g,r,h), H, H] state-side weights
    w_d = nc.dram_tensor("w", [N_LAYERS, 4, H, H], bf16, kind="ExternalInput")
    wf_d = nc.dram_tensor("wf", [H, 1], bf16, kind="ExternalInput")
    wfb_d = nc.dram_tensor("wfb", [1, 1], f32, kind="ExternalInput")
    out_d = nc.dram_tensor("out", [1, B_CORE], f32, kind="ExternalOutput")

    bufs_small = int(os.environ.get("K_BUFS_SMALL", "8"))
    bufs_zgr_sb = int(os.environ.get("K_BUFS_ZGRSB", "3"))
    bufs_xa = int(os.environ.get("K_BUFS_XA", "6"))
    bufs_ps_zgr = int(os.environ.get("K_BUFS_PSZGR", "2"))
    bufs_ps_h = int(os.environ.get("K_BUFS_PSH", "2"))

    with tile.TileContext(nc) as tc:
        with (
            tc.tile_pool(name="consts", bufs=1) as cp,
            tc.tile_pool(name="small", bufs=bufs_small) as sp,
            tc.tile_pool(name="zgrsb", bufs=bufs_zgr_sb) as zp,
            tc.tile_pool(name="xap", bufs=bufs_xa) as xp,
            tc.tile_pool(name="ps_zgr", bufs=bufs_ps_zgr, space="PSUM") as pz,
            tc.tile_pool(name="ps_small", bufs=bufs_ps_h, space="PSUM") as pm,
        ):
            t_sw = cp.tile([KA, H], bf16)
            nc.sync.dma_start(out=t_sw[:], in_=sw_d[:])
            # U weights: strip j (partitions 32j..32j+8) holds gate ORDER4[j]
            ORDER4 = [2, 0, 1, 3]   # psum slots: r, z, g, h
            t_u4 = cp.tile([H, N_LAYERS * H], bf16)
            t_w = cp.tile([H, N_LAYERS * 4 * H], bf16)
            for l in range(N_LAYERS):
                for j, g in enumerate(ORDER4):
                    nc.sync.dma_start(
                        out=t_u4[32 * j:32 * j + KA, l * H:(l + 1) * H],
                        in_=u_d[l, g])
                for g in range(4):
                    off = (l * 4 + g) * H
                    nc.sync.dma_start(out=t_w[:, off:off + H], in_=w_d[l, g])
            t_wf = cp.tile([H, 1], bf16)
            nc.sync.dma_start(out=t_wf[:], in_=wf_d[:])
            t_wfb = cp.tile([1, 1], f32)
            nc.sync.dma_start(out=t_wfb[:], in_=wfb_d[:])

            def w_ap(layer, gate):
                off = (layer * 4 + gate) * H
                return t_w[:, off:off + H]

            ABL = os.environ.get("K_ABL", "")
            if ABL == "mm_only":
                acf = cp.tile([H, N], f32)
                nc.vector.memset(acf[:], 0.5)
                a_const = cp.tile([H, N], bf16)
                nc.vector.tensor_copy(a_const[:], acf[:])
            state = {}

            def stage_load(t):
                xa = xp.tile([H, N], bf16, tag="xa")
                for j in range(4):
                    nc.sync.dma_start(out=xa[32 * j:32 * j + KA, :],
                                      in_=xa_d[:, t * N:(t + 1) * N])
                state[t] = {"xa": xa, "s": None, "ps_h": None}

            def stage_layer(t, l):
                st = state[t]
                xa = st["xa"]
                if l == 0:
                    ps_s0 = pm.tile([H, N], f32, tag="h")
                    nc.tensor.matmul(ps_s0[:], t_sw[:], xa[0:KA, :],
                                     start=True, stop=True)
                s_prev = st["s"]

                if ABL == "mm_only":
                    a = a_const
                else:
                    a = sp.tile([H, N], bf16, tag="a")
                    nc.scalar.activation(a[:], ps_s0[:] if l == 0 else s_prev[:], SIG)

                # gate order in psum slots: r, z, g (r first: its sigmoid can
                # split out early to start the H chain)
                ps_zgr = pz.tile([H, 3 * N], f32, tag="zgr")
                ps_h = pm.tile([H, N], f32, tag="h")
                order = [2, 0, 1]
                targets = [ps_zgr[:, 0:N], ps_zgr[:, N:2 * N],
                           ps_zgr[:, 2 * N:3 * N], ps_h[:]]
                for j in range(4):
                    nc.tensor.matmul(targets[j],
                                     t_u4[32 * j:32 * j + KA, l * H:(l + 1) * H],
                                     xa[32 * j:32 * j + KA, :],
                                     start=True, stop=False,
                                     tile_position=(32 * j, 0))
                for slot, g in enumerate(order):
                    nc.tensor.matmul(ps_zgr[:, slot * N:(slot + 1) * N],
                                     w_ap(l, g), a[:], start=False, stop=True)

                zgr = zp.tile([H, 3 * N], f32, tag="zgr_sb")
                if ABL == "mm_only":
                    pass
                elif os.environ.get("K_SPLIT_R", "0") == "1":
                    nc.scalar.activation(zgr[:, 0:N], ps_zgr[:, 0:N], SIG)
                    nc.scalar.activation(zgr[:, N:3 * N], ps_zgr[:, N:3 * N], SIG)
                else:
                    nc.scalar.activation(zgr[:], ps_zgr[:], SIG)
                r_sl = zgr[:, 0:N]
                z_sl = zgr[:, N:2 * N]
                g_sl = zgr[:, 2 * N:3 * N]

                if ABL == "mm_only":
                    sr = a_const
                else:
                    sr = sp.tile([H, N], bf16, tag="sr")
                    nc.vector.tensor_mul(sr[:], a[:], r_sl)
                nc.tensor.matmul(ps_h[:], w_ap(l, 3), sr[:],
                                 start=False, stop=True)

                if ABL == "mm_only":
                    st["s"] = a_const
                    st["ps_h"] = ps_h
                    return
                hh = sp.tile([H, N], f32, tag="hh")
                nc.scalar.activation(hh[:], ps_h[:], TANH)

                t1 = sp.tile([H, N], f32, tag="t1")
                nc.vector.tensor_mul(t1[:], g_sl, hh[:])        # (1-G)*H
                t2 = sp.tile([H, N], f32, tag="t2")
                nc.gpsimd.tensor_mul(t2[:], z_sl, a[:])         # Z*A
                if l < N_LAYERS - 1:
                    s_new = sp.tile([H, N], f32, tag="s")
                else:
                    # last layer: S feeds only the final matmul -> bf16
                    s_new = sp.tile([H, N], bf16, tag="s3")
                nc.vector.tensor_add(s_new[:], t1[:], t2[:])
                st["s"] = s_new
                st["ps_h"] = ps_h

            def stage_out(t):
                st = state.pop(t)
                s3, ps_h = st["s"], st["ps_h"]
                # final projection rides in rows [0:1] of the (consumed) ps_h
                ps_out = ps_h[0:1, :]
                nc.tensor.matmul(ps_out, t_wf[:], s3[:], start=True, stop=True)
                o_sb = sp.tile([1, N], f32, tag="o_sb")
                nc.vector.tensor_scalar_add(o_sb[:], ps_out, t_wfb[:])
                nc.sync.dma_start(out=out_d[0:1, t * N:(t + 1) * N], in_=o_sb[:])

            # software-pipelined emission, deepest stage first
            SPC = int(os.environ.get("K_SPACING", "2"))
            DEPTH = 1 + N_LAYERS * SPC + 1

            def body(_=None):
                for k in range(N_TILES + DEPTH):
                    if 0 <= k - DEPTH:
                        stage_out(k - DEPTH)
                    for l in reversed(range(N_LAYERS)):
                        ti = k - 1 - l * SPC
                        if 0 <= ti < N_TILES:
                            stage_layer(ti, l)
                    if k < N_TILES:
                        stage_load(k)

            if reps == 1:
                body()
            else:
                with tc.For_i(0, reps, 1) as _i:
                    body(_i)

    nc.compile()
    return nc


def _prep(inputs):
    x = np.asarray(inputs["x"], np.float32)
    xaug = np.empty((KA, B), np.float32)
    xaug[0] = 1.0
    xaug[1:] = x.T

    sw = np.concatenate(
        [np.asarray(inputs["Sw_b"], np.float32)[None, :],
         np.asarray(inputs["Sw_w"], np.float32)], axis=0)

    u = np.empty((N_LAYERS, 4, KA, H), np.float32)
    w = np.empty((N_LAYERS, 4, H, H), np.float32)
    gates_u = ["Uz", "Ug", "Ur", "Uh"]
    gates_wb = [("Wsz_w", "Wsz_b"), ("Wsg_w", "Wsg_b"),
                ("Wsr_w", "Wsr_b"), ("Wsh_w", "Wsh_b")]
    for g in range(4):
        u[:, g, 0, :] = np.asarray(inputs[gates_wb[g][1]], np.float32)
        u[:, g, 1:, :] = np.asarray(inputs[gates_u[g]], np.float32)
        w[:, g] = np.asarray(inputs[gates_wb[g][0]], np.float32)
    # negate the G gate so sigmoid(pre) = 1 - G
    u[:, 1] *= -1.0
    w[:, 1] *= -1.0

    wf = np.asarray(inputs["Wf_w"], np.float32).reshape(H, 1)
    wfb = np.asarray(inputs["Wf_b"], np.float32).reshape(1, 1)

    b = ml_dtypes.bfloat16
    return (xaug.astype(b), sw.astype(b), u.astype(b), w.astype(b),
            wf.astype(b), wfb)


def kernel(**inputs):
    if "nc" not in _CACHE:
        _CACHE["nc"] = _build()
    nc = _CACHE["nc"]

    xaug, sw, u, w, wf, wfb = _prep(inputs)
    in_maps = []
    for c in range(N_CORES):
        in_maps.append({
            "xa": np.ascontiguousarray(xaug[:, c * B_CORE:(c + 1) * B_CORE]),
            "sw": sw, "u": u, "w": w, "wf": wf, "wfb": wfb,
        })
    res = run_bass_kernel_spmd(nc, in_maps, core_ids=list(range(N_CORES)))
    out = np.concatenate([res.results[c]["out"][0] for c in range(N_CORES)])
    return out.reshape(B, 1).astype(np.float32)
